# revision 1
# baseline (speedup 1.0000x reference)
import numpy as np
import jax
import jax.numpy as jnp
from jax.sharding import Mesh, PartitionSpec as P
from jax.experimental.shard_map import shard_map

# Problem constants (hardcoded per spec)
N = 50000      # nodes
E = 800000     # edges
IN = 256       # in_feats
H = 256        # hidden
HEADS = 4
DH = H // HEADS
SCALE = np.sqrt(DH).astype(np.float32)
NCORES = 8
B = N // NCORES          # 6250 nodes per core
CH_SIZE = 6272           # edges per scan chunk (multiple of 128)

_cache = {}


def _device_fn(feats, ns_full, nd_loc, src_c, dst_c, mask_c, Wm, bm,
               WQ1, bQ1, WK1, bK1, WV1, bV1,
               WQ2, bQ2, WK2, bK2, WV2, bV2,
               WQ3, bQ3, WK3, bK3, WV3, bV3,
               W1, b1, W2, b2, W3, b3):
    nd_loc = nd_loc[0]        # [B,1]
    src_c = src_c[0]          # [NCH, CH]  padded src -> N (zero row)
    dst_c = dst_c[0]          # [NCH, CH]  local dst, padded -> 0
    mask_c = mask_c[0]        # [NCH, CH]

    x = jax.nn.relu(feats @ Wm + bm)          # [N,H] replicated
    zrow = jnp.zeros((1, H), jnp.float32)

    def layer(x_full, WQ, bQ, WK, bK, WV, bV):
        xn_ext = jnp.concatenate([x_full * ns_full, zrow])   # [N+1,H]

        def gcn_step(carry, inp):
            s, d = inp
            m = xn_ext[s]                                    # [CH,H]
            return carry + jax.ops.segment_sum(m, d, num_segments=B), None

        agg, _ = jax.lax.scan(gcn_step, jnp.zeros((B, H), jnp.float32),
                              (src_c, dst_c))
        aggn = agg * nd_loc
        Q_loc = jax.nn.relu(aggn @ WQ + bQ).reshape(B, HEADS, DH)
        K_loc = jax.nn.relu(aggn @ WK + bK)
        V_loc = jax.nn.relu(aggn @ WV + bV)
        K_ext = jnp.concatenate([jax.lax.all_gather(K_loc, 'x', tiled=True), zrow])
        V_ext = jnp.concatenate([jax.lax.all_gather(V_loc, 'x', tiled=True), zrow])

        def att_step(carry, inp):
            s, d, mk = inp
            Ke = K_ext[s].reshape(-1, HEADS, DH)
            Qe = Q_loc[d]
            sc = jnp.exp(jnp.clip((Ke * Qe).sum(-1) / SCALE, -10.0, 10.0))
            sc = sc * mk[:, None]                            # [CH,HEADS]
            Ve = V_ext[s].reshape(-1, HEADS, DH)
            wv = jax.ops.segment_sum(Ve * sc[:, :, None], d, num_segments=B)
            zz = jax.ops.segment_sum(sc, d, num_segments=B)
            return (carry[0] + wv, carry[1] + zz), None

        (wV, z), _ = jax.lax.scan(
            att_step,
            (jnp.zeros((B, HEADS, DH), jnp.float32),
             jnp.zeros((B, HEADS), jnp.float32)),
            (src_c, dst_c, mask_c))
        x_loc = (wV / (z[:, :, None] + 1e-6)).reshape(B, H)
        x_next = jax.lax.all_gather(x_loc, 'x', tiled=True)
        return x_loc, x_next

    x1_loc, x1 = layer(x, WQ1, bQ1, WK1, bK1, WV1, bV1)
    x2_loc, x2 = layer(x1, WQ2, bQ2, WK2, bK2, WV2, bV2)
    x3_loc, _ = layer(x2, WQ3, bQ3, WK3, bK3, WV3, bV3)

    xc = jnp.concatenate((x1_loc, x2_loc, x3_loc), axis=1)
    h = jax.nn.relu(xc @ W1 + b1)
    h = jax.nn.relu(h @ W2 + b2)
    out_loc = jax.nn.sigmoid((h @ W3 + b3)[:, 0])
    return out_loc[None]


def _build():
    if 'fn' in _cache:
        return _cache['fn']
    mesh = Mesh(np.array(jax.devices()[:NCORES]), ('x',))
    specs_in = (P(), P(), P('x'), P('x'), P('x'), P('x')) + (P(),) * 26
    fn = jax.jit(shard_map(_device_fn, mesh=mesh,
                           in_specs=specs_in, out_specs=P('x'),
                           check_rep=False))
    _cache['fn'] = fn
    return fn


def _prep(src, dst):
    deg_out = np.bincount(src, minlength=N).astype(np.float32)
    deg_in = np.bincount(dst, minlength=N).astype(np.float32)
    ns = np.where(deg_out > 0, deg_out ** -0.5, 0.0).astype(np.float32)[:, None]
    nd = np.where(deg_in > 0, deg_in ** -0.5, 0.0).astype(np.float32)[:, None]
    part = dst // B
    order = np.argsort(part, kind='stable')
    src_s, dst_s, part_s = src[order], dst[order], part[order]
    counts = np.bincount(part_s, minlength=NCORES)
    nch = int((counts.max() + CH_SIZE - 1) // CH_SIZE)
    Epc = nch * CH_SIZE
    src_sh = np.full((NCORES, Epc), N, np.int32)      # pad -> zero row
    dst_sh = np.zeros((NCORES, Epc), np.int32)        # pad -> 0 (masked)
    mask_sh = np.zeros((NCORES, Epc), np.float32)
    off = 0
    for c in range(NCORES):
        n = int(counts[c])
        src_sh[c, :n] = src_s[off:off + n]
        dst_sh[c, :n] = dst_s[off:off + n] - c * B
        mask_sh[c, :n] = 1.0
        off += n
    return (ns, nd.reshape(NCORES, B, 1),
            src_sh.reshape(NCORES, nch, CH_SIZE),
            dst_sh.reshape(NCORES, nch, CH_SIZE),
            mask_sh.reshape(NCORES, nch, CH_SIZE))


def _kernel_numpy(features, src, dst, W):
    # pure-host fallback, exact mirror of the reference
    deg_out = np.bincount(src, minlength=N).astype(np.float32)
    deg_in = np.bincount(dst, minlength=N).astype(np.float32)
    ns = np.where(deg_out > 0, deg_out ** -0.5, 0.0)[:, None].astype(np.float32)
    nd = np.where(deg_in > 0, deg_in ** -0.5, 0.0)[:, None].astype(np.float32)
    relu = lambda a: np.maximum(a, 0.0)

    def gcn(x):
        m = (x * ns)[src]
        agg = np.zeros((N, x.shape[1]), np.float32)
        np.add.at(agg, dst, m)
        return agg * nd

    x = relu(features @ W['Wm'] + W['bm'])
    outs = []
    for l in (1, 2, 3):
        agg = gcn(x)
        Q = relu(agg @ W[f'WQ{l}'] + W[f'bQ{l}']).reshape(N, HEADS, DH)
        K = relu(agg @ W[f'WK{l}'] + W[f'bK{l}']).reshape(N, HEADS, DH)
        V = relu(agg @ W[f'WV{l}'] + W[f'bV{l}']).reshape(N, HEADS, DH)
        sc = np.exp(np.clip((K[src] * Q[dst]).sum(-1) / SCALE, -10.0, 10.0))
        wV = np.zeros((N, HEADS, DH), np.float32)
        np.add.at(wV, dst, V[src] * sc[:, :, None])
        z = np.zeros((N, HEADS), np.float32)
        np.add.at(z, dst, sc)
        x = (wV / (z[:, :, None] + 1e-6)).reshape(N, H).astype(np.float32)
        outs.append(x)
    xc = np.concatenate(outs, axis=1)
    h = relu(xc @ W['W1'] + W['b1'])
    h = relu(h @ W['W2'] + W['b2'])
    o = (h @ W['W3'] + W['b3'])[:, 0]
    return (1.0 / (1.0 + np.exp(-o))).astype(np.float32)


def kernel(features, src, dst, edge_types, Wm, bm,
           WQ1, bQ1, WK1, bK1, WV1, bV1,
           WQ2, bQ2, WK2, bK2, WV2, bV2,
           WQ3, bQ3, WK3, bK3, WV3, bV3,
           W1, b1, W2, b2, W3, b3, **_unused):
    features = np.asarray(features, np.float32)
    src = np.asarray(src).astype(np.int64)
    dst = np.asarray(dst).astype(np.int64)
    W = {k: np.asarray(v, np.float32) for k, v in dict(
        Wm=Wm, bm=bm, WQ1=WQ1, bQ1=bQ1, WK1=WK1, bK1=bK1, WV1=WV1, bV1=bV1,
        WQ2=WQ2, bQ2=bQ2, WK2=WK2, bK2=bK2, WV2=WV2, bV2=bV2,
        WQ3=WQ3, bQ3=bQ3, WK3=WK3, bK3=bK3, WV3=WV3, bV3=bV3,
        W1=W1, b1=b1, W2=W2, b2=b2, W3=W3, b3=b3).items()}
    try:
        ns, nd_sh, src_sh, dst_sh, mask_sh = _prep(src, dst)
        fn = _build()
        out = fn(features, ns, nd_sh, src_sh, dst_sh, mask_sh,
                 W['Wm'], W['bm'],
                 W['WQ1'], W['bQ1'], W['WK1'], W['bK1'], W['WV1'], W['bV1'],
                 W['WQ2'], W['bQ2'], W['WK2'], W['bK2'], W['WV2'], W['bV2'],
                 W['WQ3'], W['bQ3'], W['WK3'], W['bK3'], W['WV3'], W['bV3'],
                 W['W1'], W['b1'], W['W2'], W['b2'], W['W3'], W['b3'])
        out = np.asarray(out).reshape(N)
        if not np.all(np.isfinite(out)):
            raise RuntimeError("non-finite device output")
        return out
    except Exception:
        return _kernel_numpy(features, src, dst, W)

